# revision 48
# baseline (speedup 1.0000x reference)
"""GAT (2-layer, global-softmax attention) Trainium2 Bass kernel, 8-core SPMD.

Sharding: core c owns batch c//4 and DEST-node block i0 = 128*(c%4). Each
core computes e[i_shard, j] for its 128 attention rows against all N=512
source nodes, the masked exp, and its own output rows
U[i_shard, m] = sum_j E[i,j] h[j, m] — the aggregation needs NO collective.
The only cross-core data is one AllGather (4-core group) per layer
boundary carrying U1^T bf16 (301 rows: three m-chunks + the
softmax-denominator partial in the last row). A 1-row dummy AllGather
issued at t~0 absorbs the CC stream's ~11us first-op setup and warms the
transfer path, all concurrent with layer-1 compute. Everything downstream
is linear in U1, so w1, a1I, a1J and biases are host-folded and the 1/S1
scale rides activation `scale` operands. Layer 2's denominator partials go
back to the host, which sums and divides during unsharding.

Key structure (vs a naive port):
- No hT/XBAR transposes anywhere: h1/h2 are computed directly in [j, m]
  layout (lhsT = fT / gathered-U1T chunks), sj1 comes from the host-folded
  wsj = w0 @ a1J~, and the layer-1 aggregation emits U1^T directly
  (lhsT = h1 m-chunk, rhs = ET1). Value path stays bf16 (fp8 there costs
  ~2% output error — only attention operands siT/sjT ride fp8).
- h1/h2 projection matmuls are interleaved into the score-slab stream as
  PE side work; const DMAs are class-ordered so the first score slab
  fires at ~12us; output DMA is bf16, split across queues.
- epilogue: per 256-col half, e = accP - accN then exp(leaky(e)) then ONE
  STT (mult adj, accum_out rowsum) — no +30/-30 mask dance. A dummy Exp
  at t~7us pre-warms the ScalarE activation table so the epilogue's first
  Exp doesn't stall 1.3us on ACT_TABLE_LOAD (function switches reload the
  table — this is also why leaky-relu stays on DVE, not AF.Lrelu). All S-path
  fold trees are absorbed into the V accumulators mid-stream (pos acc is
  memset to a2b), and a few V units are reserved for the stream tail so
  the absorbs overlap production.

Edge scores: with z = relu(s_i[i,k] + s_j[j,k] + b[k]),
e[i,j] = sum_k z[i,j,k]*a2[k]. |a2[k]| is folded into the projections and
k sorted positive-signs-first. Per k a rank-2 TensorE matmul
([siT_k; 1]^T @ [1; sjT_k], fp8 DoubleRow) produces a (128,512) f32 slab
in PSUM, consumed by two parallel paths:
  S-path: ScalarE relu on slab PAIRS -> bf16 tree tiles, folded
          incrementally (GpSimd pair-chase + DVE combines + GpSimd absorb);
  V-path: DVE scalar_tensor_tensor acc = relu(z) + acc (f32).

(A peer-to-peer SWDGE remote-DMA exchange was tried instead of the
AllGather; data routing worked but the descriptor-ring cost ~48us for
75KB — the collective is faster here.)
"""

import sys

if "/opt/trn_rl_repo" not in sys.path:
    sys.path.insert(0, "/opt/trn_rl_repo")

import numpy as np
import ml_dtypes

import concourse.bass as bass
import concourse.mybir as mybir
import concourse.tile as tile
from concourse import bacc
from concourse.bass_utils import run_bass_kernel_spmd

BF16 = mybir.dt.bfloat16
F32 = mybir.dt.float32
FP8 = mybir.dt.float8e4
DR = mybir.MatmulPerfMode.DoubleRow
AF = mybir.ActivationFunctionType
ALU = mybir.AluOpType

B, N, IN_DIM, MEM, HID = 2, 512, 512, 300, 64
P = 128  # i-shard rows per core
NCORES = 8
GROUPS = [[0, 1, 2, 3], [4, 5, 6, 7]]
NEG_SLOPE = 0.01

MC = [128, 128, 44]  # chunks of MEM=300
NJC = N // P  # 4 j-blocks

# consume mix per layer: slabs to Scalar relu + tree folds vs DVE STT.
X_S, X_V = 42, 22



def _consume_assignment(p_pos):
    """Split k in [0,64) (pos sign first) into per-engine lists and an
    interleaved unit schedule: ('S', sign, k0, k1) pairs, ('V', sign, k).
    The last few V units per sign are reserved for the very end of the
    stream so every S-path fold tree completes (and is absorbed into the
    V accumulator) while PE still produces slabs — the tail combine is
    then a single subtract."""
    units_by_sign = []
    for sign, ks in ((1, list(range(p_pos))), (0, list(range(p_pos, HID)))):
        n = len(ks)
        s_n = min(n, int(round(X_S * n / HID / 2.0)) * 2)
        su = [("S", sign, ks[2 * t], ks[2 * t + 1]) for t in range(s_n // 2)]
        vu = [("V", sign, k) for k in ks[s_n:]]
        n_res = min(2, len(vu))
        res = vu[len(vu) - n_res :]
        vu = vu[: len(vu) - n_res]
        merged = []
        iters = [su, vu]
        tot = sum(len(x) for x in iters)
        idx = [0.0, 0.0]
        for _ in range(tot):
            best = max(
                (0, 1),
                key=lambda q: (len(iters[q]) - idx[q]) / max(len(iters[q]), 1),
            )
            merged.append(iters[best][int(idx[best])])
            idx[best] += 1
        units_by_sign.append((merged, res, s_n, n - s_n))
    (mu0, r0, s0, v0), (mu1, r1, s1, v1) = units_by_sign
    merged = []
    i0 = i1 = 0
    t0, t1 = len(mu0), len(mu1)
    while i0 < t0 or i1 < t1:
        if i1 >= t1 or (i0 < t0 and i0 * t1 <= i1 * t0):
            merged.append(mu0[i0])
            i0 += 1
        else:
            merged.append(mu1[i1])
            i1 += 1
    # reserved V units, alternating signs, at the absolute end
    tail = []
    for a, b in zip(r0, r1):
        tail.extend([a, b])
    tail.extend(r0[len(r1) :])
    tail.extend(r1[len(r0) :])
    merged.extend(tail)
    counts = {"s_pos": s0, "v_pos": v0, "s_neg": s1, "v_neg": v1}
    return merged, counts


def _tree_fold(nc, eng, tile_, nslab):
    """Fold nslab bf16 slabs (contiguous [128, nslab, 512]) down to slab 0
    with wide contiguous adds on `eng`. Returns AP of the folded slab."""
    n = nslab
    while n > 1:
        lo = n - n // 2
        w = n - lo
        eng.tensor_add(
            tile_[:, 0:w, :], tile_[:, 0:w, :], tile_[:, lo : lo + w, :]
        )
        n = lo
    return tile_[:, 0, :]


def _emit_scores(nc, pools, cst, lay, lhsJ, rhsA, sched, counts, a2b, adjt,
                 side=None):
    """Produce + consume the 64 score slabs; epilogue to E=exp + rowsums.
    side: optional {unit_idx: callable} PE side-work injected into the
    stream. Returns (E bf16 [128,512], sE f32 [128,1])."""
    work, zp2, zp1 = pools["work"], pools["zp2"], pools["zp1"]
    side = side or {}

    tree = {}
    fill = {}
    for sign in (0, 1):
        ns = counts["s_pos" if sign else "s_neg"]
        tiles = []
        for t in range((ns + 7) // 8):
            cap = min(8, ns - 8 * t)
            tiles.append(
                (
                    work.tile(
                        [128, 8, 512], BF16, tag=f"tr{sign}{t}",
                        name=f"tr{sign}{t}_{lay}",
                    ),
                    cap,
                )
            )
        tree[sign] = tiles
        fill[sign] = 0
    # V-path accumulators, created upfront. The pos accumulator starts at
    # a2b so the tail combine is a bare subtract.
    accs = {}
    for sign in (0, 1):
        at = work.tile(
            [128, 512], F32, tag=f"accV{sign}", name=f"accV{sign}_{lay}"
        )
        accs[sign] = at
        nc.vector.memset(at[:, :], float(a2b) if sign == 1 else 0.0)

    def acc_consume(sign, zslab):
        at = accs[sign]
        nc.vector.scalar_tensor_tensor(
            at[:, :], zslab, 0.0, at[:, :], op0=ALU.max, op1=ALU.add
        )

    def absorb(sign, tt):
        # fold tree tile slab 0 into the V accumulator (mid-stream, on
        # GpSimd — DVE is saturated by the V-path STTs)
        nc.gpsimd.tensor_add(accs[sign][:, :], accs[sign][:, :], tt[:, 0, :])

    def maybe_fold(sign):
        """Tile-0 of each sign folds on GpSimd with fine-grained pair adds
        that chase the relu stream; other tiles fold on DVE when complete.
        Completed tiles are absorbed into the V accumulator immediately."""
        pos = fill[sign]
        t = (pos - 1) // 8
        tt, cap = tree[sign][t]
        if t <= 1 and cap == 8:
            sl = ((pos - 1) % 8) - 1  # first slot of the pair just written
            if sl == 2:
                nc.gpsimd.tensor_add(tt[:, 0, :], tt[:, 0, :], tt[:, 1, :])
            if sl >= 2:
                nc.gpsimd.tensor_add(
                    tt[:, sl, :], tt[:, sl, :], tt[:, sl + 1, :]
                )
            if pos == t * 8 + 8:  # complete: combine pair sums on DVE
                nc.vector.tensor_add(tt[:, 0, :], tt[:, 0, :], tt[:, 2, :])
                nc.vector.tensor_add(tt[:, 4, :], tt[:, 4, :], tt[:, 6, :])
                nc.vector.tensor_add(tt[:, 0, :], tt[:, 0, :], tt[:, 4, :])
                absorb(sign, tt)
        elif pos == t * 8 + cap:
            _tree_fold(nc, nc.vector, tt, cap)
            absorb(sign, tt)

    for ui, unit in enumerate(sched):
        if ui in side:
            side[ui]()
        path, sign = unit[0], unit[1]
        if path == "S":
            k0, k1 = unit[2], unit[3]
            z = zp2.tile([128, 2, 512], F32, tag="z2")
            nc.tensor.matmul(
                z[:, 0, :], lhsJ[0:1, k0, :, :], rhsA[0:1, k0, :, :],
                start=True, stop=True, perf_mode=DR,
            )
            nc.tensor.matmul(
                z[:, 1, :], lhsJ[0:1, k1, :, :], rhsA[0:1, k1, :, :],
                start=True, stop=True, perf_mode=DR,
            )
            pos = fill[sign]
            t, slot = pos // 8, pos % 8
            tt, cap = tree[sign][t]
            nc.scalar.activation(tt[:, slot : slot + 2, :], z[:, :, :], AF.Relu)
            fill[sign] = pos + 2
            maybe_fold(sign)
        else:
            k = unit[2]
            z = zp1.tile([128, 512], F32, tag="z1")
            nc.tensor.matmul(
                z[:, :], lhsJ[0:1, k, :, :], rhsA[0:1, k, :, :],
                start=True, stop=True, perf_mode=DR,
            )
            acc_consume(sign, z[:, :])
    for ui in sorted(side):
        if ui >= len(sched):
            side[ui]()

    # j-half pipelined epilogue: every fold tree was already absorbed into
    # the V accumulators mid-stream (pos started at a2b), so per half:
    # subtract -> leaky relu -> exp -> mask-multiply (+fused rowsum accum).
    e = work.tile([128, 512], F32, tag="ecomb", name=f"ecomb_{lay}")
    lr = work.tile([128, 512], F32, tag="lr", name=f"lr_{lay}")
    Eun = work.tile([128, 512], BF16, tag="Eun", name=f"Eun_{lay}")
    E = work.tile([128, 512], BF16, tag="E", name=f"E_{lay}")
    sE2 = work.tile([128, 2], F32, tag="sE2", name=f"sE2_{lay}")
    for h in (0, 1):
        sl = slice(h * 256, (h + 1) * 256)
        nc.vector.tensor_sub(e[:, sl], accs[1][:, sl], accs[0][:, sl])
        nc.vector.scalar_tensor_tensor(
            lr[:, sl], e[:, sl], NEG_SLOPE, e[:, sl], op0=ALU.mult, op1=ALU.max
        )
        nc.scalar.activation(Eun[:, sl], lr[:, sl], AF.Exp)
        nc.vector.scalar_tensor_tensor(
            E[:, sl], Eun[:, sl], 1.0, adjt[:, sl], op0=ALU.mult, op1=ALU.mult,
            accum_out=sE2[:, h : h + 1],
        )
    sE = work.tile([128, 1], F32, tag="sE", name=f"sE_{lay}")
    nc.vector.tensor_add(sE[:, :], sE2[:, 0:1], sE2[:, 1:2])
    return E, sE


def _emit_ET(nc, pools, cst, lay, E):
    """Transpose E via PE (idle at the layer tail) + Scalar psum->sbuf copy."""
    work, mp = pools["work"], pools["mp"]
    ET = work.tile([128, NJC, 128], BF16, tag="ET", name=f"ET_{lay}")
    for jb in range(NJC):
        pt = mp.tile([128, 512], BF16, tag="mm", name=f"etp{jb}_{lay}")
        nc.tensor.transpose(
            pt[:, :128], E[:, jb * 128 : (jb + 1) * 128], cst["identt"][:, :]
        )
        nc.scalar.activation(ET[:, jb, :], pt[:, :128], AF.Copy, bias=0.0)
    return ET


def _emit_sum_partial(nc, pools, cst, lay, sE):
    work, mp = pools["work"], pools["mp"]
    sEb = work.tile([128, 1], BF16, tag="sEb", name=f"sEb_{lay}")
    nc.vector.tensor_copy(sEb[:, :], sE[:, :])
    ps = mp.tile([128, 512], F32, tag="mm", name=f"sS_{lay}")
    nc.tensor.matmul(
        ps[:1, :1], sEb[:, 0:1], cst["onest"][:, 0:1], start=True, stop=True
    )
    sp = work.tile([1, 1], BF16, tag="sp", name=f"sp_{lay}")
    nc.vector.tensor_copy(sp[:, :], ps[:1, :1])
    return sp


def _build(p_pos, a2b, debug, dbg_taps=False):
    sched, counts = _consume_assignment(p_pos)
    nc = bacc.Bacc(
        "TRN2",
        target_bir_lowering=False,
        debug=debug,
        num_devices=NCORES,
    )
    d_dbg = {}
    if dbg_taps:
        for nm, shp, dt in [
            ("dbg_E1", [128, 512], BF16), ("dbg_E2", [128, 512], BF16),
            ("dbg_h1", [128, 4 * 300], BF16),
        ]:
            d_dbg[nm] = nc.dram_tensor(nm, shp, dt, kind="ExternalOutput")

    d_fT = nc.dram_tensor("fT", [128, 4 * N], BF16, kind="ExternalInput")
    d_fT8 = nc.dram_tensor("fT8", [128, 4 * N], FP8, kind="ExternalInput")
    d_fTo8 = nc.dram_tensor("fTo8", [128, 4 * P], FP8, kind="ExternalInput")
    d_adj = nc.dram_tensor("adjm", [P, N], BF16, kind="ExternalInput")
    d_w0 = nc.dram_tensor("w0b", [128, 4 * 300], BF16, kind="ExternalInput")
    d_wsi = nc.dram_tensor("wsib", [128, 4 * 64], FP8, kind="ExternalInput")
    d_wsj = nc.dram_tensor("wsjb", [128, 4 * 64], FP8, kind="ExternalInput")
    d_csi = nc.dram_tensor("csic", [64, 1], F32, kind="ExternalInput")
    d_csj = nc.dram_tensor("csjc", [64, 1], F32, kind="ExternalInput")
    d_b0B = nc.dram_tensor("b0B", [128, 300], BF16, kind="ExternalInput")
    d_w1 = nc.dram_tensor("w1b", [128, 3 * 300], BF16, kind="ExternalInput")
    d_wsi2 = nc.dram_tensor("wsi2b", [128, 3 * 64], BF16, kind="ExternalInput")
    d_wsj2 = nc.dram_tensor("wsj2b", [128, 3 * 64], BF16, kind="ExternalInput")
    d_csi2 = nc.dram_tensor("csi2c", [64, 1], F32, kind="ExternalInput")
    d_csj2 = nc.dram_tensor("csj2c", [64, 1], F32, kind="ExternalInput")
    d_b1B = nc.dram_tensor("b1B", [128, 300], BF16, kind="ExternalInput")
    d_ones = nc.dram_tensor("onesb", [1, HID * 512], FP8, kind="ExternalInput")
    d_id = nc.dram_tensor("ident", [128, 128], BF16, kind="ExternalInput")
    d_outU = nc.dram_tensor("outU", [P, 300], BF16, kind="ExternalOutput")
    d_outS = nc.dram_tensor("outS", [1, 1], F32, kind="ExternalOutput")

    with tile.TileContext(nc) as tc:
        with (
            tc.tile_pool(name="const", bufs=1) as const,
            tc.tile_pool(name="work", bufs=1) as work,
            tc.tile_pool(name="mp", bufs=2, space="PSUM") as mp,
            tc.tile_pool(name="zp2", bufs=2, space="PSUM") as zp2,
            tc.tile_pool(name="zp1", bufs=2, space="PSUM") as zp1,
            tc.tile_pool(name="dram", bufs=1, space="DRAM") as dram,
        ):
            pools = {"work": work, "mp": mp, "zp2": zp2, "zp1": zp1}

            zrow = const.tile([1, 128], BF16, tag="zrow")
            nc.vector.memset(zrow[:, :], 0.0)
            # dummy 1-row collective: pays the CC stream's first-op setup
            # and warms the transfer path concurrently with layer-1 compute
            dum_in = dram.tile([1, 128], BF16, tag="dumin")
            dum_out = dram.tile([4, 128], BF16, tag="dumout")
            nc.sync.dma_start(out=dum_in[:, :], in_=zrow[:, :])
            nc.gpsimd.collective_compute(
                "AllGather", ALU.bypass, replica_groups=GROUPS,
                ins=[dum_in.opt()], outs=[dum_out.opt()],
            )

            # ---- const loads: class-1 (gates the first score slab at
            # t~8us: wsi/fTo/wsj/ones/fT) spread evenly over the 3 DMA
            # queues; everything else queued after fT ----
            # DoubleRow fp8 operands first — the ones rows gate the very
            # first score matmul, so they lead their queues
            lhsJ = work.tile([1, HID, 2, 128], FP8, tag="lhsJ")
            rhsA = work.tile([1, HID, 2, 512], FP8, tag="rhsA")
            nc.gpsimd.dma_start(out=rhsA[0:1, :, 0, :], in_=d_ones[0:1, :])
            nc.sync.dma_start(
                out=lhsJ[0:1, :, 1, :], in_=d_ones[0:1, 0 : HID * 128]
            )
            wsit = const.tile([128, 4, 64], FP8, tag="wsit")
            nc.sync.dma_start(wsit[:, :, :], d_wsi[:, :])
            fTo = const.tile([128, 4, 128], FP8, tag="fTo")
            nc.scalar.dma_start(fTo[:, :, :], d_fTo8[:, :])
            wsjt = const.tile([128, 4, 64], FP8, tag="wsjt")
            nc.gpsimd.dma_start(wsjt[:, :, :], d_wsj[:, :])
            csic = const.tile([64, 1], F32, tag="csic")
            nc.gpsimd.dma_start(csic[:, :], d_csi[:, :])
            csjc = const.tile([64, 1], F32, tag="csjc")
            nc.gpsimd.dma_start(csjc[:, :], d_csj[:, :])

            # fT8: the critical-path load (sjT1 needs all 4 chunks) — the
            # fp8 attention-path copy is half the bytes of the bf16 one,
            # so scores start ~7us earlier; the bf16 copy (h1 value path)
            # streams in behind it.
            fT8 = const.tile([128, 4, 512], FP8, tag="fT8")
            fT = const.tile([128, 4, 512], BF16, tag="fT")
            w0t = const.tile([128, 4, 300], BF16, tag="w0t")
            dma_engs = [nc.sync, nc.scalar, nc.gpsimd]
            for hs in range(4):
                dma_engs[hs % 3].dma_start(
                    fT8[:, hs, :], d_fT8[:, hs * 512 : (hs + 1) * 512]
                )

            # ================= LAYER 1 =================
            # siT1[k, i'] = wsi^T fTo + csi  (host-folded)
            siT1 = work.tile([64, 128], FP8, tag="siT1")
            ps = mp.tile([128, 512], F32, tag="mm", name="siT1p")
            for kt in range(4):
                nc.tensor.matmul(
                    ps[:64, :128], wsit[:, kt, :], fTo[:, kt, :],
                    start=(kt == 0), stop=(kt == 3),
                )
            nc.scalar.activation(
                siT1[:, :], ps[:64, :128], AF.Identity, bias=csic[:, :],
                scale=1.0 / 16.0,
            )
            nc.scalar.dma_start(out=lhsJ[0:1, :, 0, :], in_=siT1[:, :])

            # sjT1[k, j] = wsj^T fT + csj  (host-folded; no hT1!)
            sjT1 = work.tile([64, 512], FP8, tag="sjT1")
            ps = mp.tile([128, 512], F32, tag="mm", name="sjT1p")
            for kt in range(4):
                nc.tensor.matmul(
                    ps[:64, :], wsjt[:, kt, :], fT8[:, kt, :],
                    start=(kt == 0), stop=(kt == 3),
                )
            nc.scalar.activation(
                sjT1[:, :], ps[:64, :], AF.Identity, bias=csjc[:, :],
                scale=1.0 / 16.0,
            )
            nc.sync.dma_start(
                out=rhsA[0:1, 0:32, 1, :], in_=sjT1[0:32, :]
            )
            nc.scalar.dma_start(
                out=rhsA[0:1, 32:64, 1, :], in_=sjT1[32:64, :]
            )

            # bf16 fT + w0 (h1 value path, needed from ~unit-18 side work)
            # load AFTER the latency-critical lhsJ/rhsA writes
            for hs in range(8):
                kt, half = hs // 2, hs % 2
                dma_engs[hs % 3].dma_start(
                    fT[:, kt, half * 256 : (half + 1) * 256],
                    d_fT[:, kt * 512 + half * 256 : kt * 512 + (half + 1) * 256],
                )
            for kt in range(4):
                dma_engs[(kt + 1) % 3].dma_start(
                    w0t[:, kt, :], d_w0[:, kt * 300 : (kt + 1) * 300]
                )

            # ---- class-2 consts (needed from ~15us on), behind the
            # latency-critical lhsJ/rhsA writes in every queue ----
            b0Bt = const.tile([128, 300], BF16, tag="b0Bt")
            nc.sync.dma_start(b0Bt[:, :], d_b0B[:, :])
            adjt = const.tile([128, 512], BF16, tag="adjt")
            nc.gpsimd.dma_start(adjt[:, :], d_adj[:, :])
            identt = const.tile([128, 128], BF16, tag="identt")
            nc.scalar.dma_start(identt[:, :], d_id[:, :])
            w1t = const.tile([128, 3, 300], BF16, tag="w1t")
            nc.gpsimd.dma_start(w1t[:, :, :], d_w1[:, :])
            wsi2t = const.tile([128, 3, 64], BF16, tag="wsi2t")
            nc.scalar.dma_start(wsi2t[:, :, :], d_wsi2[:, :])
            wsj2t = const.tile([128, 3, 64], BF16, tag="wsj2t")
            nc.sync.dma_start(wsj2t[:, :, :], d_wsj2[:, :])
            csi2c = const.tile([64, 1], F32, tag="csi2c")
            nc.gpsimd.dma_start(csi2c[:, :], d_csi2[:, :])
            csj2c = const.tile([64, 1], F32, tag="csj2c")
            nc.gpsimd.dma_start(csj2c[:, :], d_csj2[:, :])
            b1Bt = const.tile([128, 300], BF16, tag="b1Bt")
            nc.scalar.dma_start(b1Bt[:, :], d_b1B[:, :])
            onest = const.tile([128, 128], BF16, tag="onest")
            nc.vector.memset(onest[:, :], 1.0)
            expw = const.tile([1, 1], F32, tag="expw")
            nc.scalar.activation(expw[:, :], onest[0:1, 0:1], AF.Exp)
            cst = dict(onest=onest, identt=identt, zrow=zrow)

            # h1[j, m] direct (j-block partition layout)
            h1 = work.tile([128, NJC, 300], BF16, tag="h1")

            def h1_side(jb):
                def emit():
                    pj = mp.tile([128, 512], F32, tag="mm", name=f"h1p{jb}")
                    for kt in range(4):
                        nc.tensor.matmul(
                            pj[:, :300],
                            fT[:, kt, jb * 128 : (jb + 1) * 128],
                            w0t[:, kt, :],
                            start=(kt == 0), stop=(kt == 3),
                        )
                    nc.vector.tensor_add(h1[:, jb, :], pj[:, :300], b0Bt[:, :])
                return emit

            side1 = {18: h1_side(0), 24: h1_side(1), 30: h1_side(2),
                     36: h1_side(3)}
            E1, sE1 = _emit_scores(
                nc, pools, cst, 0, lhsJ, rhsA, sched, counts, a2b, adjt,
                side=side1,
            )
            if dbg_taps:
                nc.sync.dma_start(out=d_dbg["dbg_E1"][:, :], in_=E1[:, :])
                nc.sync.dma_start(out=d_dbg["dbg_h1"][:, :], in_=h1[:, :, :])
            ET1 = _emit_ET(nc, pools, cst, 0, E1)
            sp1 = _emit_sum_partial(nc, pools, cst, 0, sE1)

            # U1^T directly: U1T[m, i] = sum_jb h1[j, m]^T @ ET1[j, i]
            U1Tq = work.tile([128, 2, 128], BF16, tag="U1Tq")
            U1Tr = work.tile([44, 128], BF16, tag="U1Tr")
            for mc in range(3):
                msz, mo = MC[mc], mc * 128
                pt = mp.tile([128, 512], F32, tag="mm", name=f"u1t{mc}")
                for jb in range(NJC):
                    nc.tensor.matmul(
                        pt[:msz, :128],
                        h1[:, jb, mo : mo + msz],
                        ET1[:, jb, :],
                        start=(jb == 0), stop=(jb == NJC - 1),
                    )
                if mc < 2:
                    nc.scalar.activation(
                        U1Tq[:, mc, :], pt[:msz, :128], AF.Copy, bias=0.0
                    )
                else:
                    nc.scalar.activation(
                        U1Tr[:, :], pt[:msz, :128], AF.Copy, bias=0.0
                    )

            # gather payload: one collective [301, 128] bf16 = U1T chunks +
            # the S1 partial in the last row (one CC op — each op carries
            # ~5-8us of fixed stream overhead, so don't split)
            ccin = dram.tile([301, 128], BF16, tag="ccin")
            ccout = dram.tile([4 * 301, 128], BF16, tag="ccout")
            nc.sync.dma_start(out=ccin[300:301, :], in_=zrow[:, :])
            nc.sync.dma_start(out=ccin[0:128, :], in_=U1Tq[:, 0, :])
            nc.scalar.dma_start(out=ccin[128:256, :], in_=U1Tq[:, 1, :])
            nc.sync.dma_start(out=ccin[256:300, :], in_=U1Tr[:, :])
            nc.sync.dma_start(out=ccin[300:301, 0:1], in_=sp1[:, :])

            nc.gpsimd.collective_compute(
                "AllGather", ALU.bypass, replica_groups=GROUPS,
                ins=[ccin.opt()], outs=[ccout.opt()],
            )

            # si2 raw from own U1T (pre-gather): si2[i, k] = U1T^T wsi2
            ps2 = mp.tile([128, 512], F32, tag="mm", name="si2p")
            nc.tensor.matmul(
                ps2[:128, :64], U1Tq[:, 0, :], wsi2t[:, 0, :],
                start=True, stop=False,
            )
            nc.tensor.matmul(
                ps2[:128, :64], U1Tq[:, 1, :], wsi2t[:, 1, :],
                start=False, stop=False,
            )
            nc.tensor.matmul(
                ps2[:128, :64], U1Tr[:, :], wsi2t[:44, 2, :],
                start=False, stop=True,
            )
            si2o = work.tile([128, 128], BF16, tag="si2o")
            nc.vector.tensor_copy(si2o[:, 0:64], ps2[:128, :64])
            siT2r = work.tile([128, 128], BF16, tag="siT2r")
            nc.scalar.dma_start_transpose(out=siT2r[:, :], in_=si2o[:, :])
            siT2 = work.tile([64, 128], FP8, tag="siT2")

            # ---- post-gather ----
            sS4 = work.tile([4, 1], BF16, tag="sS4")
            nc.sync.dma_start(out=sS4[:, :], in_=ccout[300 : 4 * 301 : 301, 0:1])
            U1gq = work.tile([128, 2, 512], BF16, tag="U1gq")
            qeng = [nc.sync, nc.gpsimd]
            for mc in range(2):
                for s in range(4):
                    qeng[(mc * 4 + s) % 2].dma_start(
                        out=U1gq[:, mc, s * 128 : (s + 1) * 128],
                        in_=ccout[s * 301 + mc * 128 : s * 301 + mc * 128 + 128, :],
                    )
            U1gr = work.tile([44, 512], BF16, tag="U1gr")
            for s in range(4):
                qeng[s % 2].dma_start(
                    out=U1gr[:, s * 128 : (s + 1) * 128],
                    in_=ccout[s * 301 + 256 : s * 301 + 300, :],
                )
            psS = mp.tile([128, 512], F32, tag="mm", name="psS1")
            nc.tensor.matmul(
                psS[:128, 0:1], onest[0:4, :], sS4[:, :], start=True, stop=True
            )
            rS1 = work.tile([128, 1], F32, tag="rS1")
            nc.vector.reciprocal(rS1[:, :], psS[:128, 0:1])

            # ================= LAYER 2 =================
            # sjT2 = (wsj2^T U1g) * rS1 + csj2
            sjT2 = work.tile([64, 512], FP8, tag="sjT2")
            ps = mp.tile([128, 512], F32, tag="mm", name="sjT2p")
            nc.tensor.matmul(
                ps[:64, :], wsj2t[:, 0, :], U1gq[:, 0, :],
                start=True, stop=False,
            )
            nc.tensor.matmul(
                ps[:64, :], wsj2t[:, 1, :], U1gq[:, 1, :],
                start=False, stop=False,
            )
            nc.tensor.matmul(
                ps[:64, :], wsj2t[:44, 2, :], U1gr[:, :],
                start=False, stop=True,
            )
            nc.scalar.activation(
                sjT2[:, :], ps[:64, :], AF.Identity,
                bias=csj2c[:, :], scale=rS1[:64, :],
            )
            nc.scalar.dma_start(out=rhsA[0:1, :, 1, :], in_=sjT2[:, :])

            # siT2: raw from own U1T (pre-gather); only the rS1 scale +
            # flatten wait on the collective.
            nc.scalar.activation(
                siT2[:, :], siT2r[:64, :], AF.Identity,
                bias=csi2c[:, :], scale=rS1[:64, :],
            )
            nc.scalar.dma_start(out=lhsJ[0:1, :, 0, :], in_=siT2[:, :])

            # h2[j, m] direct from gathered U1T chunks, interleaved into
            # the L2 score stream
            h2sc = work.tile([128, NJC, 300], BF16, tag="h2sc")

            def h2_side(jb):
                def emit():
                    pj = mp.tile([128, 512], F32, tag="mm", name=f"h2p{jb}")
                    nc.tensor.matmul(
                        pj[:, :300],
                        U1gq[:, 0, jb * 128 : (jb + 1) * 128],
                        w1t[:, 0, :],
                        start=True, stop=False,
                    )
                    nc.tensor.matmul(
                        pj[:, :300],
                        U1gq[:, 1, jb * 128 : (jb + 1) * 128],
                        w1t[:, 1, :],
                        start=False, stop=False,
                    )
                    nc.tensor.matmul(
                        pj[:, :300],
                        U1gr[:, jb * 128 : (jb + 1) * 128],
                        w1t[:44, 2, :],
                        start=False, stop=True,
                    )
                    nc.vector.scalar_tensor_tensor(
                        h2sc[:, jb, :], pj[:, :300], rS1[:, :], b1Bt[:, :],
                        op0=ALU.mult, op1=ALU.add,
                    )
                return emit

            side2 = {1: h2_side(0), 4: h2_side(1), 7: h2_side(2),
                     10: h2_side(3)}
            E2, sE2 = _emit_scores(
                nc, pools, cst, 1, lhsJ, rhsA, sched, counts, a2b, adjt,
                side=side2,
            )
            if dbg_taps:
                nc.sync.dma_start(out=d_dbg["dbg_E2"][:, :], in_=E2[:, :])
            ET2 = _emit_ET(nc, pools, cst, 1, E2)
            sp2 = _emit_sum_partial(nc, pools, cst, 1, sE2)
            sp2f = work.tile([1, 1], F32, tag="sp2f")
            nc.vector.tensor_copy(sp2f[:, :], sp2[:, :])
            nc.gpsimd.dma_start(out=d_outS[:, :], in_=sp2f[:, :])
            pu2 = mp.tile([128, 512], F32, tag="mm", name="U_1")
            for jb in range(NJC):
                nc.tensor.matmul(
                    pu2[:, :MEM], ET2[:, jb, :], h2sc[:, jb, :],
                    start=(jb == 0), stop=(jb == NJC - 1),
                )
            stout = work.tile([128, 300], BF16, tag="stout")
            nc.scalar.activation(stout[:, 0:150], pu2[:, 0:150], AF.Copy, bias=0.0)
            nc.sync.dma_start(out=d_outU[:, 0:75], in_=stout[:, 0:75])
            nc.gpsimd.dma_start(out=d_outU[:, 75:150], in_=stout[:, 75:150])
            nc.scalar.activation(
                stout[:, 150:300], pu2[:, 150:MEM], AF.Copy, bias=0.0
            )
            nc.sync.dma_start(out=d_outU[:, 150:225], in_=stout[:, 150:225])
            nc.scalar.dma_start(out=d_outU[:, 225:300], in_=stout[:, 225:300])

    nc.compile()
    return nc


_CACHE = {}


def _get_program(p_pos, a2b, debug=False, dbg_taps=False):
    key = (p_pos, float(a2b), debug, dbg_taps)
    if key not in _CACHE:
        _CACHE[key] = _build(p_pos, float(a2b), debug, dbg_taps=dbg_taps)
    return _CACHE[key]


def _pack_tiles(arr, nkt):
    """(rows, w) -> (128, nkt*w): row t*128+p lands at [p, t*w:(t+1)*w],
    zero-padding rows to nkt*128."""
    rows, w = arr.shape
    padded = np.zeros((nkt * 128, w), np.float32)
    padded[:rows] = arr
    return np.ascontiguousarray(
        padded.reshape(nkt, 128, w).transpose(1, 0, 2).reshape(128, nkt * w)
    )


def _prep_inputs(feature, adj, w0, b0, w1, b1, a1_w, a1_b, a2_w, a2_b):
    """Host-side packing: dtype casts, |a2| fold, sign sort, weight folds,
    shard slices."""
    bf = ml_dtypes.bfloat16
    a2 = np.asarray(a2_w, np.float32).reshape(-1)
    order = np.argsort((a2 < 0).astype(np.int32), kind="stable")
    p_pos = int((a2 >= 0).sum())
    absa2 = np.abs(a2[order])
    a1s = np.asarray(a1_w, np.float32)[:, order] * absa2[None, :]  # (600, 64)
    a1bs = np.asarray(a1_b, np.float32)[order] * absa2  # (64,)

    w0f = np.asarray(w0, np.float32)
    w1f = np.asarray(w1, np.float32)
    b0f = np.asarray(b0, np.float32)
    b1f = np.asarray(b1, np.float32)

    f8 = ml_dtypes.float8_e4m3fn
    w0b = _pack_tiles(w0f, 4).astype(bf)
    wsib = _pack_tiles(w0f @ a1s[:MEM] * 16.0, 4).astype(f8)
    csi = (b0f @ a1s[:MEM])[:, None].astype(np.float32)
    wsjb = _pack_tiles(w0f @ a1s[MEM:] * 16.0, 4).astype(f8)
    csj = (b0f @ a1s[MEM:] + a1bs)[:, None].astype(np.float32)

    w1b = _pack_tiles(w1f, 3).astype(bf)
    wsi2b = _pack_tiles(w1f @ a1s[:MEM], 3).astype(bf)
    wsj2b = _pack_tiles(w1f @ a1s[MEM:], 3).astype(bf)
    csi2 = (b1f @ a1s[:MEM])[:, None].astype(np.float32)
    csj2 = (b1f @ a1s[MEM:] + a1bs)[:, None].astype(np.float32)
    b0B = np.broadcast_to(b0f[None, :], (128, MEM)).astype(bf).copy()
    b1B = np.broadcast_to(b1f[None, :], (128, MEM)).astype(bf).copy()
    onesb = np.ones((1, HID * 512), np.float32).astype(ml_dtypes.float8_e4m3fn)
    ident = np.eye(128, dtype=np.float32).astype(bf)

    featT = [np.asarray(feature[b], np.float32).T for b in range(B)]
    fTb = [_pack_tiles(featT[b], 4).astype(bf) for b in range(B)]
    fT8b = [_pack_tiles(featT[b], 4).astype(f8) for b in range(B)]
    adjf = np.asarray(adj, np.float32)
    in_maps = []
    for c in range(NCORES):
        b, i0 = c // 4, 128 * (c % 4)
        fTo8 = _pack_tiles(featT[b][:, i0 : i0 + P], 4).astype(f8)
        adjm = np.ascontiguousarray(adjf[b][i0 : i0 + P, :]).astype(bf)
        in_maps.append(
            {
                "fT": fTb[b],
                "fT8": fT8b[b],
                "fTo8": fTo8,
                "adjm": adjm,
                "w0b": w0b,
                "wsib": wsib,
                "wsjb": wsjb,
                "csic": csi,
                "csjc": csj,
                "b0B": b0B,
                "w1b": w1b,
                "wsi2b": wsi2b,
                "wsj2b": wsj2b,
                "csi2c": csi2,
                "csj2c": csj2,
                "b1B": b1B,
                "onesb": onesb,
                "ident": ident,
            }
        )
    a2b = float(np.asarray(a2_b, np.float32).reshape(-1)[0])
    return in_maps, p_pos, a2b


def kernel(feature, adj, w0, b0, w1, b1, a1_w, a1_b, a2_w, a2_b, _trace=False,
           _debug=False, _dbg_taps=False):
    in_maps, p_pos, a2b = _prep_inputs(
        feature, adj, w0, b0, w1, b1, a1_w, a1_b, a2_w, a2_b
    )
    nc = _get_program(p_pos, a2b, debug=_debug, dbg_taps=_dbg_taps)
    res = run_bass_kernel_spmd(
        nc, in_maps, core_ids=list(range(NCORES)), trace=_trace
    )
    out = np.zeros((B, N, MEM), np.float32)
    for b in range(B):
        s = sum(
            float(np.asarray(res.results[4 * b + g]["outS"], np.float32)[0, 0])
            for g in range(4)
        )
        for g in range(4):
            u = np.asarray(res.results[4 * b + g]["outU"]).astype(np.float32)
            out[b, 128 * g : 128 * (g + 1), :] = u / s
    kernel._last_exec_time_ns = res.exec_time_ns
    kernel._last_profile = res.profile_json
    kernel._last_results = res
    return out
